# revision 4
# baseline (speedup 1.0000x reference)
"""Trainium2 Bass kernel for nn_AttentionLayer (label-wise attention pooling).

Reference computation (per batch b):
    w   = tanh(x @ W1^T)                    [L, DA]
    lg  = w @ W2^T                          [L, NL]
    att = softmax(lg, axis=L)               [L, NL]
    attT = att^T                            [NL, L]   (output)
    ctx = attT @ x                          [NL, D]   (output)
    wout = rowsum(ctx * W3) + b3            [NL]      (output)

Strategy: pure data-parallel over batch across 8 NeuronCores (4 batches per
core), weights replicated.  Per core, per batch, stream over L in chunks of
512.  All matmuls contract over the partition dim, so x must be transposed
(PE-transpose) for the first matmul; logits are computed directly in the
transposed [NL, L] layout so the softmax reduction lands on the free dim and
attT is produced in its output layout.  exp() is kept unnormalized; the
transposed-back E (PE-transpose) feeds the context matmul, and both outputs
are scaled by 1/S at the end (softmax normalization commutes with the
contraction over L).
"""

import os
import sys

import numpy as np

sys.path.insert(0, "/opt/trn_rl_repo")

import concourse.bass as bass  # noqa: E402
import concourse.mybir as mybir  # noqa: E402
import concourse.tile as tile  # noqa: E402
from concourse import bacc  # noqa: E402
from concourse.bass_utils import run_bass_kernel_spmd  # noqa: E402
from concourse.masks import make_identity  # noqa: E402

B, L, D, DA, NL = 32, 2048, 512, 512, 512
NCORES = 8
BPC = B // NCORES  # batches per core
P = 128
LC = 512  # l-chunk size
NCHUNK = L // LC
NSUB = LC // P  # l-subtiles per chunk
KD = D // P  # d k-tiles
KA = DA // P  # a k-tiles
NT = NL // P  # n tiles

F32 = mybir.dt.float32
F32R = mybir.dt.float32r

AF = mybir.ActivationFunctionType
ALU = mybir.AluOpType
AX = mybir.AxisListType

# matmul compute dtype: float32r runs the PE at full (bf16) rate for moving
# free dim >= 256; plain float32 runs at 1/4 rate.  Toggle for accuracy A/B.
MM_DT = F32R if os.environ.get("KERNEL_MM_DT", "f32r") == "f32r" else F32


def _mm(ap):
    return ap.bitcast(MM_DT) if MM_DT != F32 else ap


def build_nc():
    nc = bacc.Bacc(None, target_bir_lowering=False, debug=False)

    x_d = nc.declare_dram_parameter("x", [BPC, L, D], F32, isOutput=False)
    w1_d = nc.declare_dram_parameter("W1", [DA, D], F32, isOutput=False)
    w2_d = nc.declare_dram_parameter("W2", [NL, DA], F32, isOutput=False)
    w3_d = nc.declare_dram_parameter("W3", [NL, D], F32, isOutput=False)
    b3_d = nc.declare_dram_parameter("b3", [NL], F32, isOutput=False)
    ctx_d = nc.declare_dram_parameter("ctx", [BPC, NL, D], F32, isOutput=True)
    wout_d = nc.declare_dram_parameter("wout", [BPC, NL], F32, isOutput=True)
    attw_d = nc.declare_dram_parameter("attw", [BPC, NL, L], F32, isOutput=True)

    with tile.TileContext(nc) as tc:
        with (
            tc.tile_pool(name="const", bufs=1) as const_pool,
            tc.tile_pool(name="weights", bufs=1) as wpool,
            tc.tile_pool(name="chunks", bufs=2) as cpool,
            tc.tile_pool(name="et", bufs=1) as etpool,
            tc.tile_pool(name="outs", bufs=2) as opool,
            tc.tile_pool(name="psum_mm", bufs=2, space="PSUM") as psmm,
            tc.tile_pool(name="psum_tr", bufs=2, space="PSUM") as pstr,
            tc.tile_pool(name="psum_ctx", bufs=1, space="PSUM") as psctx,
        ):
            ident = const_pool.tile([P, P], F32)
            make_identity(nc, ident[:])

            # ---- weight prep (one-time) ----
            # W1T[p_d, dk, a] = W1[a, dk*P + p_d]; W2T[p_a, ak, n] = W2[n, ak*P + p_a]
            w1n = wpool.tile([P, KA, D], F32)
            nc.sync.dma_start(w1n[:], w1_d.rearrange("(o p) d -> p o d", p=P))
            w2n = wpool.tile([P, NT, DA], F32)
            nc.sync.dma_start(w2n[:], w2_d.rearrange("(o p) d -> p o d", p=P))
            w1t = wpool.tile([P, KD, DA], F32)
            w2t = wpool.tile([P, KA, NL], F32)
            for src, dst, n_o in ((w1n, w1t, KA), (w2n, w2t, NT)):
                for dk in range(KD):
                    ps = pstr.tile([P, 512], F32, tag="tr")
                    for o in range(n_o):
                        nc.tensor.transpose(
                            ps[:, o * P : (o + 1) * P],
                            src[:, o, dk * P : (dk + 1) * P],
                            ident[:],
                        )
                    nc.vector.tensor_copy(dst[:, dk, :], ps[:])
            w3sb = wpool.tile([P, NT, D], F32)
            nc.sync.dma_start(w3sb[:], w3_d.rearrange("(o p) d -> p o d", p=P))
            b3sb = wpool.tile([P, NT], F32)
            with nc.allow_non_contiguous_dma(reason="tiny 2KB bias load"):
                nc.sync.dma_start(b3sb[:], b3_d.rearrange("(o p) -> p o", p=P))

            # ---- main loop over local batches ----
            for b in range(BPC):
                e_t = etpool.tile([P, NT, L], F32, tag="ET")
                s_part = opool.tile([P, NT, NCHUNK], F32, tag="Spart")
                ps_ctx = [
                    psctx.tile([P, D], F32, tag=f"ctx{nt}", name=f"psctx{nt}")
                    for nt in range(NT)
                ]
                for c in range(NCHUNK):
                    lsl = slice(c * LC, (c + 1) * LC)
                    x_c = cpool.tile([P, NSUB, D], F32, tag="xc")
                    nc.sync.dma_start(
                        x_c[:], x_d[b, lsl, :].rearrange("(s p) d -> p s d", p=P)
                    )
                    # xT_c[p_d, dk, l] = x[c*LC + l, dk*P + p_d]
                    xt_c = cpool.tile([P, KD, LC], F32, tag="xTc")
                    for dk in range(KD):
                        ps = pstr.tile([P, LC], F32, tag="tr")
                        for s in range(NSUB):
                            nc.tensor.transpose(
                                ps[:, s * P : (s + 1) * P],
                                x_c[:, s, dk * P : (dk + 1) * P],
                                ident[:],
                            )
                        nc.vector.tensor_copy(xt_c[:, dk, :], ps[:])
                    # wT_c[p_a, at, l] = tanh(sum_d W1[a, d] * x[l, d])
                    wt_c = cpool.tile([P, KA, LC], F32, tag="wTc")
                    for at in range(KA):
                        ps = psmm.tile([P, LC], F32, tag="mm")
                        for dk in range(KD):
                            nc.tensor.matmul(
                                ps[:],
                                _mm(w1t[:, dk, at * P : (at + 1) * P]),
                                _mm(xt_c[:, dk, :]),
                                start=(dk == 0),
                                stop=(dk == KD - 1),
                            )
                        nc.scalar.activation(wt_c[:, at, :], ps[:], AF.Tanh)
                    # lgT -> E_T[p_n, nt, l] = exp(sum_a W2[n, a] * w[l, a]),
                    # accumulating per-(nt, chunk) row sums into s_part
                    for nt in range(NT):
                        ps = psmm.tile([P, LC], F32, tag="mm")
                        for ak in range(KA):
                            nc.tensor.matmul(
                                ps[:],
                                _mm(w2t[:, ak, nt * P : (nt + 1) * P]),
                                _mm(wt_c[:, ak, :]),
                                start=(ak == 0),
                                stop=(ak == KA - 1),
                            )
                        nc.scalar.activation(
                            e_t[:, nt, lsl],
                            ps[:],
                            AF.Exp,
                            accum_out=s_part[:, nt, c : c + 1],
                        )
                    # E_nat[p_l, s, n] = E[c*LC + s*P + p_l, n] via PE transpose
                    e_nat = cpool.tile([P, NSUB, NL], F32, tag="Enat")
                    for s in range(NSUB):
                        ps = pstr.tile([P, NL], F32, tag="tr")
                        for nt in range(NT):
                            nc.tensor.transpose(
                                ps[:, nt * P : (nt + 1) * P],
                                e_t[:, nt, c * LC + s * P : c * LC + (s + 1) * P],
                                ident[:],
                            )
                        nc.vector.tensor_copy(e_nat[:, s, :], ps[:])
                    # ctx_raw[n, d] += sum_l E[l, n] * x[l, d]
                    for nt in range(NT):
                        for s in range(NSUB):
                            nc.tensor.matmul(
                                ps_ctx[nt][:],
                                _mm(e_nat[:, s, nt * P : (nt + 1) * P]),
                                _mm(x_c[:, s, :]),
                                start=(c == 0 and s == 0),
                                stop=(c == NCHUNK - 1 and s == NSUB - 1),
                            )

                # ---- batch epilogue ----
                s_sum = opool.tile([P, NT], F32, tag="Ssum")
                nc.vector.tensor_reduce(s_sum[:], s_part[:], axis=AX.X, op=ALU.add)
                r = opool.tile([P, NT], F32, tag="R")
                nc.vector.reciprocal(r[:], s_sum[:])

                # attT = E_T * R (in place), stream out
                for nt in range(NT):
                    for c in range(NCHUNK):
                        lsl = slice(c * LC, (c + 1) * LC)
                        nc.vector.tensor_scalar_mul(
                            e_t[:, nt, lsl], e_t[:, nt, lsl], r[:, nt : nt + 1]
                        )
                        nc.sync.dma_start(
                            attw_d[b, nt * P : (nt + 1) * P, lsl], e_t[:, nt, lsl]
                        )

                # ctx = ctx_raw * R; wout = rowsum(ctx * W3) + b3
                ctx_sb = opool.tile([P, NT, D], F32, tag="ctxsb")
                wout_sb = opool.tile([P, NT], F32, tag="woutsb")
                for nt in range(NT):
                    nc.vector.tensor_scalar_mul(
                        ctx_sb[:, nt, :], ps_ctx[nt][:], r[:, nt : nt + 1]
                    )
                    tmp = opool.tile([P, D], F32, tag="wtmp")
                    nc.vector.tensor_mul(
                        out=tmp[:], in0=ctx_sb[:, nt, :], in1=w3sb[:, nt, :]
                    )
                    col = opool.tile([P, 1], F32, tag="wcol")
                    nc.vector.tensor_reduce(col[:], tmp[:], axis=AX.X, op=ALU.add)
                    nc.vector.tensor_add(
                        out=wout_sb[:, nt : nt + 1], in0=col[:], in1=b3sb[:, nt : nt + 1]
                    )
                nc.sync.dma_start(
                    ctx_d[b].rearrange("(o p) d -> p o d", p=P), ctx_sb[:]
                )
                with nc.allow_non_contiguous_dma(reason="tiny 2KB wout store"):
                    nc.sync.dma_start(
                        wout_d[b].rearrange("(o p) -> p o", p=P), wout_sb[:]
                    )
    nc.compile()
    return nc


def _install_ntff_hook():
    """Register the axon NTFF profile hook (image's antenv lacks axon_hooks).

    Only used when BASS_TRACE=1; failures here must never break plain runs.
    """
    try:
        import types

        try:
            from antenv import axon_hooks  # noqa: F401
        except ImportError:
            import antenv

            mod = types.ModuleType("antenv.axon_hooks")
            mod._hook = None

            def set_axon_ntff_profile_hook(h):
                mod._hook = h

            def get_axon_ntff_profile_hook():
                return mod._hook

            mod.set_axon_ntff_profile_hook = set_axon_ntff_profile_hook
            mod.get_axon_ntff_profile_hook = get_axon_ntff_profile_hook
            sys.modules["antenv.axon_hooks"] = mod
            antenv.axon_hooks = mod
        from antenv.axon_hooks import (
            get_axon_ntff_profile_hook,
            set_axon_ntff_profile_hook,
        )

        if get_axon_ntff_profile_hook() is None:
            from trn_agent_boot.trn_boot import _ntff_profile_via_ctypes

            set_axon_ntff_profile_hook(
                _ntff_profile_via_ctypes("/opt/axon/libaxon_pjrt.so")
            )

        # artifact upload needs cloud creds this container doesn't have
        import concourse.bass_utils as bu

        bu.upload_artifacts = lambda tmpdir: f"local:{tmpdir}"
    except Exception as e:  # pragma: no cover
        print(f"ntff hook install skipped: {e}", file=sys.stderr)


if os.environ.get("BASS_TRACE"):
    _install_ntff_hook()

_NC_CACHE = None
LAST_RESULT = None


def kernel(x, W1, W2, W3, b3):
    global _NC_CACHE, LAST_RESULT
    x = np.ascontiguousarray(np.asarray(x, dtype=np.float32))
    W1 = np.ascontiguousarray(np.asarray(W1, dtype=np.float32))
    W2 = np.ascontiguousarray(np.asarray(W2, dtype=np.float32))
    W3 = np.ascontiguousarray(np.asarray(W3, dtype=np.float32))
    b3 = np.ascontiguousarray(np.asarray(b3, dtype=np.float32))

    if _NC_CACHE is None:
        _NC_CACHE = build_nc()
    nc = _NC_CACHE

    in_maps = [
        dict(x=x[i * BPC : (i + 1) * BPC], W1=W1, W2=W2, W3=W3, b3=b3)
        for i in range(NCORES)
    ]
    res = run_bass_kernel_spmd(nc, in_maps, core_ids=list(range(NCORES)))
    LAST_RESULT = res
    rs = res.results
    ctx = np.concatenate([r["ctx"] for r in rs], axis=0)
    wout = np.concatenate([r["wout"] for r in rs], axis=0)
    attw = np.concatenate([r["attw"] for r in rs], axis=0)
    return ctx, wout, attw


if __name__ == "__main__":
    rng = np.random.default_rng(0)
    inputs = dict(
        x=rng.standard_normal((B, L, D), dtype=np.float32),
        W1=(rng.standard_normal((DA, D), dtype=np.float32) * 0.03),
        W2=(rng.standard_normal((NL, DA), dtype=np.float32) * 0.03),
        W3=(rng.standard_normal((NL, D), dtype=np.float32) * 0.03),
        b3=np.zeros((NL,), dtype=np.float32),
    )
    outs = kernel(**inputs)
    for o in outs:
        print(o.shape, o.dtype)


# revision 9
# speedup vs baseline: 1.9771x; 1.9771x over previous
"""Trainium2 Bass kernel for nn_AttentionLayer (label-wise attention pooling).

Reference computation (per batch b):
    w   = tanh(x @ W1^T)                    [L, DA]
    lg  = w @ W2^T                          [L, NL]
    att = softmax(lg, axis=L)               [L, NL]
    attT = att^T                            [NL, L]   (output)
    ctx = attT @ x                          [NL, D]   (output)
    wout = rowsum(ctx * W3) + b3            [NL]      (output)

Strategy: pure data-parallel over batch across 8 NeuronCores (4 batches per
core), weights replicated.  Per core, per batch, stream over L in chunks of
512.  All matmuls contract over the partition dim, so x must be transposed
(PE-transpose) for the first matmul; logits are computed directly in the
transposed [NL, L] layout so the softmax reduction lands on the free dim and
attT is produced in its output layout.  exp() is kept unnormalized; the
transposed-back E (PE-transpose) feeds the context matmul, and both outputs
are scaled by 1/S at the end (softmax normalization commutes with the
contraction over L).
"""

import os
import sys

import numpy as np

sys.path.insert(0, "/opt/trn_rl_repo")

import concourse.bass as bass  # noqa: E402
import concourse.mybir as mybir  # noqa: E402
import concourse.tile as tile  # noqa: E402
from concourse import bacc  # noqa: E402
from concourse.bass_utils import run_bass_kernel_spmd  # noqa: E402
from concourse.masks import make_identity  # noqa: E402

B, L, D, DA, NL = 32, 2048, 512, 512, 512
NCORES = 8
BPC = B // NCORES  # batches per core
P = 128
LC = 512  # l-chunk size
NCHUNK = L // LC
NSUB = LC // P  # l-subtiles per chunk
KD = D // P  # d k-tiles
KA = DA // P  # a k-tiles
NT = NL // P  # n tiles

F32 = mybir.dt.float32
F32R = mybir.dt.float32r

AF = mybir.ActivationFunctionType
ALU = mybir.AluOpType
AX = mybir.AxisListType

# matmul compute dtype: float32r runs the PE at full (bf16) rate for moving
# free dim >= 256; plain float32 runs at 1/4 rate.  float32r is a rounded
# format: every producer feeding an f32r matmul must emit an f32r-typed
# output (the write port rounds).  Toggle for accuracy A/B.
MM_DT = F32R if os.environ.get("KERNEL_MM_DT", "f32r") == "f32r" else F32


def build_nc():
    nc = bacc.Bacc(None, target_bir_lowering=False, debug=False)

    x_d = nc.declare_dram_parameter("x", [BPC, L, D], F32, isOutput=False)
    w1_d = nc.declare_dram_parameter("W1", [DA, D], F32, isOutput=False)
    w2_d = nc.declare_dram_parameter("W2", [NL, DA], F32, isOutput=False)
    w3_d = nc.declare_dram_parameter("W3", [NL, D], F32, isOutput=False)
    b3_d = nc.declare_dram_parameter("b3", [NL], F32, isOutput=False)
    ctx_d = nc.declare_dram_parameter("ctx", [BPC, NL, D], F32, isOutput=True)
    wout_d = nc.declare_dram_parameter("wout", [BPC, NL], F32, isOutput=True)
    attw_d = nc.declare_dram_parameter("attw", [BPC, NL, L], F32, isOutput=True)

    with tile.TileContext(nc) as tc:
        with (
            tc.tile_pool(name="const", bufs=1) as const_pool,
            tc.tile_pool(name="weights", bufs=1) as wpool,
            tc.tile_pool(name="chunks", bufs=2) as cpool,
            tc.tile_pool(name="et", bufs=1) as etpool,
            tc.tile_pool(name="outs", bufs=2) as opool,
            tc.tile_pool(name="psum_mm", bufs=2, space="PSUM") as psmm,
            tc.tile_pool(name="psum_tr", bufs=2, space="PSUM") as pstr,
            tc.tile_pool(name="psum_ctx", bufs=1, space="PSUM") as psctx,
        ):
            ident = const_pool.tile([P, P], F32)
            make_identity(nc, ident[:])
            if MM_DT != F32:
                ident_r = const_pool.tile([P, P], MM_DT)
                nc.vector.tensor_copy(ident_r[:], ident[:])
            else:
                ident_r = ident

            # ---- weight prep (one-time) ----
            # W1T[p_d, dk, a] = W1[a, dk*P + p_d]; W2T[p_a, ak, n] = W2[n, ak*P + p_a]
            w1n = wpool.tile([P, KA, D], F32)
            nc.sync.dma_start(w1n[:], w1_d.rearrange("(o p) d -> p o d", p=P))
            w2n = wpool.tile([P, NT, DA], F32)
            nc.sync.dma_start(w2n[:], w2_d.rearrange("(o p) d -> p o d", p=P))
            w1t = wpool.tile([P, KD, DA], MM_DT)
            w2t = wpool.tile([P, KA, NL], MM_DT)
            for src, dst, n_o in ((w1n, w1t, KA), (w2n, w2t, NT)):
                for dk in range(KD):
                    ps = pstr.tile([P, 512], F32, tag="tr")
                    for o in range(n_o):
                        nc.tensor.transpose(
                            ps[:, o * P : (o + 1) * P],
                            src[:, o, dk * P : (dk + 1) * P],
                            ident[:],
                        )
                    nc.vector.tensor_copy(dst[:, dk, :], ps[:])
            w3sb = wpool.tile([P, NT, D], F32)
            nc.sync.dma_start(w3sb[:], w3_d.rearrange("(o p) d -> p o d", p=P))
            b3sb = wpool.tile([P, NT], F32)
            with nc.allow_non_contiguous_dma(reason="tiny 2KB bias load"):
                nc.sync.dma_start(b3sb[:], b3_d.rearrange("(o p) -> p o", p=P))

            # ---- main loop over local batches ----
            for b in range(BPC):
                e_t = etpool.tile([P, NT, L], F32, tag="ET")
                s_part = opool.tile([P, NT, NCHUNK], F32, tag="Spart")
                ps_ctx = [
                    psctx.tile([P, D], F32, tag=f"ctx{nt}", name=f"psctx{nt}")
                    for nt in range(NT)
                ]
                for c in range(NCHUNK):
                    lsl = slice(c * LC, (c + 1) * LC)
                    x_raw = cpool.tile([P, NSUB, D], F32, tag="xraw")
                    nc.sync.dma_start(
                        x_raw[:],
                        x_d[b, lsl, :].rearrange("(s p) d -> p s d", p=P),
                    )
                    if MM_DT == F32:
                        x_c = x_raw
                    else:
                        x_c = cpool.tile([P, NSUB, D], MM_DT, tag="xc")
                        nc.vector.tensor_copy(x_c[:], x_raw[:])
                    # xT_c[p_d, dk, l] = x[c*LC + l, dk*P + p_d]
                    xt_c = cpool.tile([P, KD, LC], MM_DT, tag="xTc")
                    for dk in range(KD):
                        ps = pstr.tile([P, LC], F32, tag="tr")
                        for s in range(NSUB):
                            nc.tensor.transpose(
                                ps[:, s * P : (s + 1) * P],
                                x_raw[:, s, dk * P : (dk + 1) * P],
                                ident[:],
                            )
                        nc.vector.tensor_copy(xt_c[:, dk, :], ps[:])
                    # wT_c[p_a, at, l] = tanh(sum_d W1[a, d] * x[l, d])
                    wt_c = cpool.tile([P, KA, LC], MM_DT, tag="wTc")
                    for at in range(KA):
                        ps = psmm.tile([P, LC], F32, tag="mm")
                        for dk in range(KD):
                            nc.tensor.matmul(
                                ps[:],
                                w1t[:, dk, at * P : (at + 1) * P],
                                xt_c[:, dk, :],
                                start=(dk == 0),
                                stop=(dk == KD - 1),
                            )
                        nc.scalar.activation(wt_c[:, at, :], ps[:], AF.Tanh)
                    # lgT -> E_T[p_n, nt, l] = exp(sum_a W2[n, a] * w[l, a]),
                    # accumulating per-(nt, chunk) row sums into s_part
                    for nt in range(NT):
                        ps = psmm.tile([P, LC], F32, tag="mm")
                        for ak in range(KA):
                            nc.tensor.matmul(
                                ps[:],
                                w2t[:, ak, nt * P : (nt + 1) * P],
                                wt_c[:, ak, :],
                                start=(ak == 0),
                                stop=(ak == KA - 1),
                            )
                        nc.scalar.activation(
                            e_t[:, nt, lsl],
                            ps[:],
                            AF.Exp,
                            accum_out=s_part[:, nt, c : c + 1],
                        )
                    # E_nat[p_l, s, n] = E[c*LC + s*P + p_l, n] via PE transpose
                    e_nat = cpool.tile([P, NSUB, NL], MM_DT, tag="Enat")
                    for s in range(NSUB):
                        ps = pstr.tile([P, NL], F32, tag="tr")
                        for nt in range(NT):
                            nc.tensor.transpose(
                                ps[:, nt * P : (nt + 1) * P],
                                e_t[:, nt, c * LC + s * P : c * LC + (s + 1) * P],
                                ident[:],
                            )
                        nc.vector.tensor_copy(e_nat[:, s, :], ps[:])
                    # ctx_raw[n, d] += sum_l E[l, n] * x[l, d]
                    for nt in range(NT):
                        for s in range(NSUB):
                            nc.tensor.matmul(
                                ps_ctx[nt][:],
                                e_nat[:, s, nt * P : (nt + 1) * P],
                                x_c[:, s, :],
                                start=(c == 0 and s == 0),
                                stop=(c == NCHUNK - 1 and s == NSUB - 1),
                            )

                # ---- batch epilogue ----
                s_sum = opool.tile([P, NT], F32, tag="Ssum")
                nc.vector.tensor_reduce(s_sum[:], s_part[:], axis=AX.X, op=ALU.add)
                r = opool.tile([P, NT], F32, tag="R")
                nc.vector.reciprocal(r[:], s_sum[:])

                # attT = E_T * R (in place), stream out
                for nt in range(NT):
                    for c in range(NCHUNK):
                        lsl = slice(c * LC, (c + 1) * LC)
                        nc.vector.tensor_scalar_mul(
                            e_t[:, nt, lsl], e_t[:, nt, lsl], r[:, nt : nt + 1]
                        )
                        nc.sync.dma_start(
                            attw_d[b, nt * P : (nt + 1) * P, lsl], e_t[:, nt, lsl]
                        )

                # ctx = ctx_raw * R; wout = rowsum(ctx * W3) + b3
                ctx_sb = opool.tile([P, NT, D], F32, tag="ctxsb")
                wout_sb = opool.tile([P, NT], F32, tag="woutsb")
                for nt in range(NT):
                    nc.vector.tensor_scalar_mul(
                        ctx_sb[:, nt, :], ps_ctx[nt][:], r[:, nt : nt + 1]
                    )
                    tmp = opool.tile([P, D], F32, tag="wtmp")
                    nc.vector.tensor_mul(
                        out=tmp[:], in0=ctx_sb[:, nt, :], in1=w3sb[:, nt, :]
                    )
                    col = opool.tile([P, 1], F32, tag="wcol")
                    nc.vector.tensor_reduce(col[:], tmp[:], axis=AX.X, op=ALU.add)
                    nc.vector.tensor_add(
                        out=wout_sb[:, nt : nt + 1], in0=col[:], in1=b3sb[:, nt : nt + 1]
                    )
                nc.sync.dma_start(
                    ctx_d[b].rearrange("(o p) d -> p o d", p=P), ctx_sb[:]
                )
                with nc.allow_non_contiguous_dma(reason="tiny 2KB wout store"):
                    nc.sync.dma_start(
                        wout_d[b].rearrange("(o p) -> p o", p=P), wout_sb[:]
                    )
    nc.compile()
    return nc


def _install_ntff_hook():
    """Register the axon NTFF profile hook (image's antenv lacks axon_hooks).

    Only used when BASS_TRACE=1; failures here must never break plain runs.
    """
    try:
        import types

        try:
            from antenv import axon_hooks  # noqa: F401
        except ImportError:
            import antenv

            mod = types.ModuleType("antenv.axon_hooks")
            mod._hook = None

            def set_axon_ntff_profile_hook(h):
                mod._hook = h

            def get_axon_ntff_profile_hook():
                return mod._hook

            mod.set_axon_ntff_profile_hook = set_axon_ntff_profile_hook
            mod.get_axon_ntff_profile_hook = get_axon_ntff_profile_hook
            sys.modules["antenv.axon_hooks"] = mod
            antenv.axon_hooks = mod
        from antenv.axon_hooks import (
            get_axon_ntff_profile_hook,
            set_axon_ntff_profile_hook,
        )

        if get_axon_ntff_profile_hook() is None:
            from trn_agent_boot.trn_boot import _ntff_profile_via_ctypes

            set_axon_ntff_profile_hook(
                _ntff_profile_via_ctypes("/opt/axon/libaxon_pjrt.so")
            )

        # artifact upload needs cloud creds this container doesn't have
        import concourse.bass_utils as bu

        bu.upload_artifacts = lambda tmpdir: f"local:{tmpdir}"
    except Exception as e:  # pragma: no cover
        print(f"ntff hook install skipped: {e}", file=sys.stderr)


if os.environ.get("BASS_TRACE"):
    _install_ntff_hook()

_NC_CACHE = None
LAST_RESULT = None


def kernel(x, W1, W2, W3, b3):
    global _NC_CACHE, LAST_RESULT
    x = np.ascontiguousarray(np.asarray(x, dtype=np.float32))
    W1 = np.ascontiguousarray(np.asarray(W1, dtype=np.float32))
    W2 = np.ascontiguousarray(np.asarray(W2, dtype=np.float32))
    W3 = np.ascontiguousarray(np.asarray(W3, dtype=np.float32))
    b3 = np.ascontiguousarray(np.asarray(b3, dtype=np.float32))

    if _NC_CACHE is None:
        _NC_CACHE = build_nc()
    nc = _NC_CACHE

    in_maps = [
        dict(x=x[i * BPC : (i + 1) * BPC], W1=W1, W2=W2, W3=W3, b3=b3)
        for i in range(NCORES)
    ]
    res = run_bass_kernel_spmd(nc, in_maps, core_ids=list(range(NCORES)))
    LAST_RESULT = res
    rs = res.results
    ctx = np.concatenate([r["ctx"] for r in rs], axis=0)
    wout = np.concatenate([r["wout"] for r in rs], axis=0)
    attw = np.concatenate([r["attw"] for r in rs], axis=0)
    return ctx, wout, attw


if __name__ == "__main__":
    rng = np.random.default_rng(0)
    inputs = dict(
        x=rng.standard_normal((B, L, D), dtype=np.float32),
        W1=(rng.standard_normal((DA, D), dtype=np.float32) * 0.03),
        W2=(rng.standard_normal((NL, DA), dtype=np.float32) * 0.03),
        W3=(rng.standard_normal((NL, D), dtype=np.float32) * 0.03),
        b3=np.zeros((NL,), dtype=np.float32),
    )
    outs = kernel(**inputs)
    for o in outs:
        print(o.shape, o.dtype)


# revision 10
# speedup vs baseline: 2.6412x; 1.3359x over previous
"""Trainium2 Bass kernel for nn_AttentionLayer (label-wise attention pooling).

Reference computation (per batch b):
    w   = tanh(x @ W1^T)                    [L, DA]
    lg  = w @ W2^T                          [L, NL]
    att = softmax(lg, axis=L)               [L, NL]
    attT = att^T                            [NL, L]   (output)
    ctx = attT @ x                          [NL, D]   (output)
    wout = rowsum(ctx * W3) + b3            [NL]      (output)

Strategy: pure data-parallel over batch across 8 NeuronCores (4 batches per
core), weights replicated.  Per core, per batch, stream over L in chunks of
512.  All matmuls contract over the partition dim, so x must be transposed
(PE-transpose) for the first matmul; logits are computed directly in the
transposed [NL, L] layout so the softmax reduction lands on the free dim and
attT is produced in its output layout.  exp() is kept unnormalized; the
transposed-back E (PE-transpose) feeds the context matmul, and both outputs
are scaled by 1/S at the end (softmax normalization commutes with the
contraction over L).
"""

import os
import sys

import numpy as np

sys.path.insert(0, "/opt/trn_rl_repo")

import concourse.bass as bass  # noqa: E402
import concourse.mybir as mybir  # noqa: E402
import concourse.tile as tile  # noqa: E402
from concourse import bacc  # noqa: E402
from concourse.bass_utils import run_bass_kernel_spmd  # noqa: E402
from concourse.masks import make_identity  # noqa: E402

B, L, D, DA, NL = 32, 2048, 512, 512, 512
NCORES = 8
BPC = B // NCORES  # batches per core
P = 128
LC = 512  # l-chunk size
NCHUNK = L // LC
NSUB = LC // P  # l-subtiles per chunk
KD = D // P  # d k-tiles
KA = DA // P  # a k-tiles
NT = NL // P  # n tiles

F32 = mybir.dt.float32
F32R = mybir.dt.float32r

AF = mybir.ActivationFunctionType
ALU = mybir.AluOpType
AX = mybir.AxisListType

# matmul compute dtype: float32r runs the PE at full (bf16) rate for moving
# free dim >= 256; plain float32 runs at 1/4 rate.  float32r is a rounded
# format: every producer feeding an f32r matmul must emit an f32r-typed
# output (the write port rounds).  Toggle for accuracy A/B.
MM_DT = F32R if os.environ.get("KERNEL_MM_DT", "f32r") == "f32r" else F32


def build_nc():
    nc = bacc.Bacc(None, target_bir_lowering=False, debug=False)

    x_d = nc.declare_dram_parameter("x", [BPC, L, D], F32, isOutput=False)
    w1_d = nc.declare_dram_parameter("W1", [DA, D], F32, isOutput=False)
    w2_d = nc.declare_dram_parameter("W2", [NL, DA], F32, isOutput=False)
    w3_d = nc.declare_dram_parameter("W3", [NL, D], F32, isOutput=False)
    b3_d = nc.declare_dram_parameter("b3", [NL], F32, isOutput=False)
    ctx_d = nc.declare_dram_parameter("ctx", [BPC, NL, D], F32, isOutput=True)
    wout_d = nc.declare_dram_parameter("wout", [BPC, NL], F32, isOutput=True)
    attw_d = nc.declare_dram_parameter("attw", [BPC, NL, L], F32, isOutput=True)

    with tile.TileContext(nc) as tc:
        with (
            tc.tile_pool(name="const", bufs=1) as const_pool,
            tc.tile_pool(name="weights", bufs=1) as wpool,
            tc.tile_pool(name="chunks", bufs=2) as cpool,
            tc.tile_pool(name="et", bufs=2) as etpool,
            tc.tile_pool(name="outs", bufs=2) as opool,
            tc.tile_pool(name="psum_mm", bufs=2, space="PSUM") as psmm,
            tc.tile_pool(name="psum_tr", bufs=2, space="PSUM") as pstr,
            tc.tile_pool(name="psum_ctx", bufs=1, space="PSUM") as psctx,
        ):
            ident = const_pool.tile([P, P], F32)
            make_identity(nc, ident[:])
            if MM_DT != F32:
                ident_r = const_pool.tile([P, P], MM_DT)
                nc.vector.tensor_copy(ident_r[:], ident[:])
            else:
                ident_r = ident

            # ---- weight prep (one-time) ----
            # W1T[p_d, dk, a] = W1[a, dk*P + p_d]; W2T[p_a, ak, n] = W2[n, ak*P + p_a]
            w1n = wpool.tile([P, KA, D], F32)
            nc.sync.dma_start(w1n[:], w1_d.rearrange("(o p) d -> p o d", p=P))
            w2n = wpool.tile([P, NT, DA], F32)
            nc.sync.dma_start(w2n[:], w2_d.rearrange("(o p) d -> p o d", p=P))
            w1t = wpool.tile([P, KD, DA], MM_DT)
            w2t = wpool.tile([P, KA, NL], MM_DT)
            for src, dst, n_o in ((w1n, w1t, KA), (w2n, w2t, NT)):
                for dk in range(KD):
                    ps = pstr.tile([P, 512], F32, tag="tr")
                    for o in range(n_o):
                        nc.tensor.transpose(
                            ps[:, o * P : (o + 1) * P],
                            src[:, o, dk * P : (dk + 1) * P],
                            ident[:],
                        )
                    nc.vector.tensor_copy(dst[:, dk, :], ps[:])
            w3sb = wpool.tile([P, NT, D], F32)
            nc.gpsimd.dma_start(w3sb[:], w3_d.rearrange("(o p) d -> p o d", p=P))
            b3sb = wpool.tile([P, NT], F32)
            with nc.allow_non_contiguous_dma(reason="tiny 2KB bias load"):
                nc.gpsimd.dma_start(b3sb[:], b3_d.rearrange("(o p) -> p o", p=P))

            # ---- main loop over local batches ----
            # Software-pipelined: x-load + xT-transpose for step i+1 are
            # emitted between the lg matmuls and the E-transposes of step i,
            # so the PE has filler work while ACT computes exp(i).
            def load_x(b, c):
                x_raw = cpool.tile([P, NSUB, D], F32, tag="xraw", name="x_raw")
                nc.sync.dma_start(
                    x_raw[:],
                    x_d[b, c * LC : (c + 1) * LC, :].rearrange(
                        "(s p) d -> p s d", p=P
                    ),
                )
                return x_raw

            def make_xt(x_raw):
                # xT[p_d, dk, l] = x[c*LC + l, dk*P + p_d]
                xt_c = cpool.tile([P, KD, LC], MM_DT, tag="xTc", name="xt_c", bufs=1)
                for dk in range(KD):
                    ps = pstr.tile([P, LC], F32, tag="tr", name="ps_tr")
                    for s in range(NSUB):
                        nc.tensor.transpose(
                            ps[:, s * P : (s + 1) * P],
                            x_raw[:, s, dk * P : (dk + 1) * P],
                            ident[:],
                        )
                    nc.vector.tensor_copy(xt_c[:, dk, :], ps[:])
                if MM_DT == F32:
                    x_c = x_raw
                else:
                    x_c = cpool.tile([P, NSUB, D], MM_DT, tag="xc", name="x_c")
                    nc.vector.tensor_copy(x_c[:], x_raw[:])
                return xt_c, x_c

            flat = [(b, c) for b in range(BPC) for c in range(NCHUNK)]
            e_t = None
            s_part = None
            ps_ctx = None
            nxt = None
            for idx, (b, c) in enumerate(flat):
                lsl = slice(c * LC, (c + 1) * LC)
                if idx == 0:
                    x_raw = load_x(b, c)
                    cur = make_xt(x_raw)
                else:
                    cur = nxt
                xt_c, x_c = cur
                if c == 0:
                    e_t = etpool.tile([P, NT, L], F32, tag="ET", name="e_t")
                    s_part = opool.tile(
                        [P, NT, NCHUNK], F32, tag="Spart", name="s_part"
                    )
                    ps_ctx = [
                        psctx.tile([P, D], F32, tag=f"ctx{nt}", name=f"psctx{nt}")
                        for nt in range(NT)
                    ]
                # wT_c[p_a, at, l] = tanh(sum_d W1[a, d] * x[l, d])
                wt_c = cpool.tile([P, KA, LC], MM_DT, tag="wTc", name="wt_c", bufs=1)
                for at in range(KA):
                    ps = psmm.tile([P, LC], F32, tag="mm", name="ps_mm")
                    for dk in range(KD):
                        nc.tensor.matmul(
                            ps[:],
                            w1t[:, dk, at * P : (at + 1) * P],
                            xt_c[:, dk, :],
                            start=(dk == 0),
                            stop=(dk == KD - 1),
                        )
                    nc.scalar.activation(wt_c[:, at, :], ps[:], AF.Tanh)
                # lgT -> E_T[p_n, nt, l] = exp(sum_a W2[n, a] * w[l, a]),
                # accumulating per-(nt, chunk) row sums into s_part
                for nt in range(NT):
                    ps = psmm.tile([P, LC], F32, tag="mm", name="ps_mm")
                    for ak in range(KA):
                        nc.tensor.matmul(
                            ps[:],
                            w2t[:, ak, nt * P : (nt + 1) * P],
                            wt_c[:, ak, :],
                            start=(ak == 0),
                            stop=(ak == KA - 1),
                        )
                    nc.scalar.activation(
                        e_t[:, nt, lsl],
                        ps[:],
                        AF.Exp,
                        accum_out=s_part[:, nt, c : c + 1],
                    )
                # prefetch + transpose next step's x while ACT runs exp
                if idx + 1 < len(flat):
                    nb, nch = flat[idx + 1]
                    x_raw_n = load_x(nb, nch)
                    nxt = make_xt(x_raw_n)
                # E_nat[p_l, s, n] = E[c*LC + s*P + p_l, n] via PE transpose
                e_nat = cpool.tile(
                    [P, NSUB, NL], MM_DT, tag="Enat", name="e_nat", bufs=1
                )
                for s in range(NSUB):
                    ps = pstr.tile([P, NL], F32, tag="tr", name="ps_tr")
                    for nt in range(NT):
                        nc.tensor.transpose(
                            ps[:, nt * P : (nt + 1) * P],
                            e_t[:, nt, c * LC + s * P : c * LC + (s + 1) * P],
                            ident[:],
                        )
                    nc.vector.tensor_copy(e_nat[:, s, :], ps[:])
                # ctx_raw[n, d] += sum_l E[l, n] * x[l, d]
                for nt in range(NT):
                    for s in range(NSUB):
                        nc.tensor.matmul(
                            ps_ctx[nt][:],
                            e_nat[:, s, nt * P : (nt + 1) * P],
                            x_c[:, s, :],
                            start=(c == 0 and s == 0),
                            stop=(c == NCHUNK - 1 and s == NSUB - 1),
                        )
                if c != NCHUNK - 1:
                    continue
                # ---- batch epilogue ----
                s_sum = opool.tile([P, NT], F32, tag="Ssum")
                nc.vector.tensor_reduce(s_sum[:], s_part[:], axis=AX.X, op=ALU.add)
                r = opool.tile([P, NT], F32, tag="R")
                nc.vector.reciprocal(r[:], s_sum[:])

                # attT = E_T * R (in place), stream out; alternate DVE/ACT
                for nt in range(NT):
                    for cc in range(NCHUNK):
                        csl = slice(cc * LC, (cc + 1) * LC)
                        if (nt * NCHUNK + cc) % 2 == 0:
                            nc.vector.tensor_scalar_mul(
                                e_t[:, nt, csl], e_t[:, nt, csl], r[:, nt : nt + 1]
                            )
                        else:
                            nc.scalar.mul(
                                e_t[:, nt, csl], e_t[:, nt, csl], r[:, nt : nt + 1]
                            )
                        nc.sync.dma_start(
                            attw_d[b, nt * P : (nt + 1) * P, csl], e_t[:, nt, csl]
                        )

                # ctx = ctx_raw * R; wout = rowsum(ctx * W3) + b3
                ctx_sb = opool.tile([P, NT, D], F32, tag="ctxsb")
                wout_sb = opool.tile([P, NT], F32, tag="woutsb")
                for nt in range(NT):
                    nc.vector.tensor_scalar_mul(
                        ctx_sb[:, nt, :], ps_ctx[nt][:], r[:, nt : nt + 1]
                    )
                    tmp = opool.tile([P, D], F32, tag="wtmp")
                    nc.vector.tensor_mul(
                        out=tmp[:], in0=ctx_sb[:, nt, :], in1=w3sb[:, nt, :]
                    )
                    col = opool.tile([P, 1], F32, tag="wcol")
                    nc.vector.tensor_reduce(col[:], tmp[:], axis=AX.X, op=ALU.add)
                    nc.vector.tensor_add(
                        out=wout_sb[:, nt : nt + 1], in0=col[:], in1=b3sb[:, nt : nt + 1]
                    )
                nc.sync.dma_start(
                    ctx_d[b].rearrange("(o p) d -> p o d", p=P), ctx_sb[:]
                )
                with nc.allow_non_contiguous_dma(reason="tiny 2KB wout store"):
                    nc.sync.dma_start(
                        wout_d[b].rearrange("(o p) -> p o", p=P), wout_sb[:]
                    )
    nc.compile()
    return nc


def _install_ntff_hook():
    """Register the axon NTFF profile hook (image's antenv lacks axon_hooks).

    Only used when BASS_TRACE=1; failures here must never break plain runs.
    """
    try:
        import types

        try:
            from antenv import axon_hooks  # noqa: F401
        except ImportError:
            import antenv

            mod = types.ModuleType("antenv.axon_hooks")
            mod._hook = None

            def set_axon_ntff_profile_hook(h):
                mod._hook = h

            def get_axon_ntff_profile_hook():
                return mod._hook

            mod.set_axon_ntff_profile_hook = set_axon_ntff_profile_hook
            mod.get_axon_ntff_profile_hook = get_axon_ntff_profile_hook
            sys.modules["antenv.axon_hooks"] = mod
            antenv.axon_hooks = mod
        from antenv.axon_hooks import (
            get_axon_ntff_profile_hook,
            set_axon_ntff_profile_hook,
        )

        if get_axon_ntff_profile_hook() is None:
            from trn_agent_boot.trn_boot import _ntff_profile_via_ctypes

            set_axon_ntff_profile_hook(
                _ntff_profile_via_ctypes("/opt/axon/libaxon_pjrt.so")
            )

        # artifact upload needs cloud creds this container doesn't have
        import concourse.bass_utils as bu

        bu.upload_artifacts = lambda tmpdir: f"local:{tmpdir}"
    except Exception as e:  # pragma: no cover
        print(f"ntff hook install skipped: {e}", file=sys.stderr)


if os.environ.get("BASS_TRACE"):
    _install_ntff_hook()

_NC_CACHE = None
LAST_RESULT = None


def kernel(x, W1, W2, W3, b3):
    global _NC_CACHE, LAST_RESULT
    x = np.ascontiguousarray(np.asarray(x, dtype=np.float32))
    W1 = np.ascontiguousarray(np.asarray(W1, dtype=np.float32))
    W2 = np.ascontiguousarray(np.asarray(W2, dtype=np.float32))
    W3 = np.ascontiguousarray(np.asarray(W3, dtype=np.float32))
    b3 = np.ascontiguousarray(np.asarray(b3, dtype=np.float32))

    if _NC_CACHE is None:
        _NC_CACHE = build_nc()
    nc = _NC_CACHE

    in_maps = [
        dict(x=x[i * BPC : (i + 1) * BPC], W1=W1, W2=W2, W3=W3, b3=b3)
        for i in range(NCORES)
    ]
    res = run_bass_kernel_spmd(nc, in_maps, core_ids=list(range(NCORES)))
    LAST_RESULT = res
    rs = res.results
    ctx = np.concatenate([r["ctx"] for r in rs], axis=0)
    wout = np.concatenate([r["wout"] for r in rs], axis=0)
    attw = np.concatenate([r["attw"] for r in rs], axis=0)
    return ctx, wout, attw


if __name__ == "__main__":
    rng = np.random.default_rng(0)
    inputs = dict(
        x=rng.standard_normal((B, L, D), dtype=np.float32),
        W1=(rng.standard_normal((DA, D), dtype=np.float32) * 0.03),
        W2=(rng.standard_normal((NL, DA), dtype=np.float32) * 0.03),
        W3=(rng.standard_normal((NL, D), dtype=np.float32) * 0.03),
        b3=np.zeros((NL,), dtype=np.float32),
    )
    outs = kernel(**inputs)
    for o in outs:
        print(o.shape, o.dtype)
